# revision 40
# baseline (speedup 1.0000x reference)
"""Trainium2 Bass kernel for a pre-norm transformer encoder layer.

Problem: x(8,1024,1024) fp32; LN1 -> MHA(16 heads, hd=64) + residual;
LN2 -> FFN(4096, exact gelu) + residual.

Strategy:
- Data-parallel: one batch element per NeuronCore (8 cores, no collectives).
- Attention-side matmuls (QKV projections, AV, out-proj) in fp8e4m3 with
  perf_mode=DoubleRow (2 contraction planes per instruction). Scores in
  fp8 at bf16 rate, packed 2-heads-concurrent via PE row tiling (K=64).
  FFN stays bf16: fp8 there pushes max-err past the 2e-2 gate.
- fp8 operands pre-scaled by powers of 2 into e4m3's normal range;
  scales folded back exactly via the ACT scale arg, DVE copy multiplies,
  and the residual scalar_tensor_tensor.
- exp(scores - 2) on the scalar engine (softmax shift-invariance) keeps
  fp8 exp values << 240 (e4m3 max).
- Softmax denominator via [V | 1] augmented AV matmul (65th output row);
  normalization fused into one scalar_tensor_tensor per ctx half.
- Query-half pipelining: attention runs per query-half (c=0 then c=1).
  During the c=1 pass, out-proj / LN2 / FFN1 of query-half 0 are emitted
  as filler units between score steps, so the scalar-engine exp chain
  hides behind dense PE work (which also keeps the HAM clock-gate at
  K=8/8). Q^T/K^T for all 8 head-pairs persist in fp8 across the passes.
"""

import numpy as np
import ml_dtypes
from contextlib import ExitStack

import concourse.bass as bass
import concourse.tile as tile
import concourse.mybir as mybir
from concourse import bacc
from concourse import bass_utils

F32 = mybir.dt.float32
BF16 = mybir.dt.bfloat16
F8 = mybir.dt.float8e4
AF = mybir.ActivationFunctionType
DR = mybir.MatmulPerfMode.DoubleRow
MULT = mybir.AluOpType.mult
ADD = mybir.AluOpType.add

F8NP = ml_dtypes.float8_e4m3fn
BFNP = ml_dtypes.bfloat16

S, D, H, HD, FF = 1024, 1024, 16, 64, 4096
ST, DT, FT = S // 128, D // 128, FF // 128
EPS = 1e-5
NCORES = 8

# power-of-2 scales bringing fp8 operand std to ~1
SCL_Q = 256.0   # wq_eff has 1/sqrt(hd)=1/8 folded in: std 1/256
SCL_K = 32.0
SCL_V = 32.0
SCL_O = 32.0
SCL_CTX = 16.0  # ctx std ~0.06
QF8 = 8.0       # Q tiles hold 8*q_eff (std ~1); exp scale arg = 1/8

_CACHE = {}


def _build_program(with_bias, dbg=False):
    nc = bacc.Bacc("TRN2", target_bir_lowering=False, debug=False,
                   num_devices=NCORES)

    din = {}
    for name, shape, dt in [
        ("x", (S, D), F32),
        ("wq", (D, D), F8), ("wk", (D, D), F8), ("wv", (D, D), F8),
        ("wo", (D, D), F8),
        ("w1", (D, FF), BF16), ("w2", (FF, D), BF16),
        ("bq", (1, D), BF16), ("bk", (1, D), BF16), ("bv", (1, D), BF16),
        ("bo", (1, D), BF16), ("b1", (1, FF), BF16), ("b2", (1, D), BF16),
        ("ident", (128, 128), BF16),
        ("ones", (1, 512), BF16), ("onescol", (1, 16), F8),
    ]:
        din[name] = nc.dram_tensor(name, shape, dt, kind="ExternalInput").ap()
    d_out = nc.dram_tensor("out", (S, D), F32, kind="ExternalOutput").ap()
    ddbg = {}
    if dbg:
        for name, shape, dt in [
            ("dbg_zT", (128, 8192), F8), ("dbg_qT", (D, S), F8),
            ("dbg_kT", (D, S), F8), ("dbg_v65", (4, 128, 2080), F8),
            ("dbg_ctxT", (128, 8192), F8), ("dbg_x2", (S, D), F32),
        ]:
            ddbg[name] = nc.dram_tensor(name, shape, dt, kind="ExternalOutput").ap()

    with tile.TileContext(nc) as tc, ExitStack() as ctx:
        _body(nc, tc, ctx, din, d_out, with_bias, ddbg)
    nc.compile()
    return nc


def _body(nc, tc, ctx, din, d_out, with_bias, ddbg=None):
    ddbg = ddbg or {}
    xp = ctx.enter_context(tc.tile_pool(name="xp", bufs=3))       # x f32 transient
    x2p = ctx.enter_context(tc.tile_pool(name="x2p", bufs=8))     # x2 f32 resident
    zp = ctx.enter_context(tc.tile_pool(name="zp", bufs=7))       # z bf16 transient
    bigp = ctx.enter_context(tc.tile_pool(name="bigp", bufs=2))   # zT/ctxT f8 wide
    qkp = ctx.enter_context(tc.tile_pool(name="qkp", bufs=16))    # Q/K f8 persistent
    v65p = ctx.enter_context(tc.tile_pool(name="v65p", bufs=4))   # V65 pairs f8
    expp = ctx.enter_context(tc.tile_pool(name="expp", bufs=8))   # exp pairs f8
    mbf = ctx.enter_context(tc.tile_pool(name="mbf", bufs=24))    # z2T/gu bf16
    wp = ctx.enter_context(tc.tile_pool(name="wp", bufs=5))       # qk/w1 col stream
    wvop = ctx.enter_context(tc.tile_pool(name="wvop", bufs=4))   # wv/wo pairs f8
    w2p = ctx.enter_context(tc.tile_pool(name="w2p", bufs=7))
    outp = ctx.enter_context(tc.tile_pool(name="outp", bufs=2))
    smallp = ctx.enter_context(tc.tile_pool(name="smallp", bufs=4))
    cstp = ctx.enter_context(tc.tile_pool(name="cstp", bufs=1))
    attp = ctx.enter_context(tc.tile_pool(name="attp", bufs=1))
    scrp = ctx.enter_context(tc.tile_pool(name="scrp", bufs=2))
    rbcp = ctx.enter_context(tc.tile_pool(name="rbcp", bufs=1))
    biasp = ctx.enter_context(tc.tile_pool(name="biasp", bufs=4)) if with_bias else None

    # constants
    ident = cstp.tile([128, 128], BF16, tag="ident")
    nc.sync.dma_start(ident[:], din["ident"])
    ones = cstp.tile([1, 512], BF16, tag="ones")
    nc.sync.dma_start(ones[:], din["ones"])
    onescol = cstp.tile([1, 16], F8, tag="onescol")
    nc.sync.dma_start(onescol[:], din["onescol"])
    eps_t = cstp.tile([128, 1], F32, tag="eps")
    nc.vector.memset(eps_t[:], EPS)
    neg2_t = cstp.tile([128, 1], F32, tag="neg2")
    nc.vector.memset(neg2_t[:], -2.0)
    ones2f = cstp.tile([128, 64], F32, tag="ones2f")
    nc.vector.memset(ones2f[:], 1.0)

    def bias_slice(dsrc, lo, n):
        bt = biasp.tile([1, 512], BF16, tag="brow")
        nc.sync.dma_start(bt[0:1, 0:n], dsrc[0:1, lo:lo + n])
        return bt[0:1, 0:n]

    # ---------------- LayerNorm -> normalized z tile (token-major) ----------
    def ln_stats_z(xt):
        stats = smallp.tile([128, 2, 6], F32, tag="stats")
        nc.vector.bn_stats(stats[:, 0, :], xt[:, 0:512])
        nc.vector.bn_stats(stats[:, 1, :], xt[:, 512:1024])
        mv = smallp.tile([128, 2], F32, tag="mv")
        nc.vector.bn_aggr(mv[:], stats[:])
        std = smallp.tile([128, 1], F32, tag="std")
        nc.scalar.activation(std[:], mv[:, 1:2], AF.Sqrt, bias=eps_t[:])
        rstd = smallp.tile([128, 1], F32, tag="rstd")
        nc.vector.reciprocal(rstd[:], std[:])
        nmr = smallp.tile([128, 1], F32, tag="nmr")
        nc.vector.tensor_scalar(nmr[:], mv[:, 0:1], rstd[:], -1.0,
                                op0=MULT, op1=MULT)
        # normalize on the scalar engine: z = x*rstd + (-mu*rstd)
        zt = zp.tile([128, 1024], BF16, name=f"z_{nc.next_id()}", tag="zp")
        nc.scalar.activation(zt[:], xt[:], AF.Identity,
                             bias=nmr[:], scale=rstd[:])
        return zt

    def transpose_half(z_tiles, h, zT_dst, ps_pool, on_scalar=False,
                       tag="pst"):
        """Transpose token-half h of 4 z tiles into zT_dst[:, j, h*512:]."""
        for j in range(DT):
            pt = ps_pool.tile([128, 512], BF16, tag=tag)
            for tl in range(4):
                t = h * 4 + tl
                nc.tensor.transpose(pt[:, tl * 128:(tl + 1) * 128],
                                    z_tiles[t][:, j * 128:(j + 1) * 128],
                                    ident[:])
            if isinstance(zT_dst, list):
                dst = zT_dst[j][:, h * 512:(h + 1) * 512]
            else:
                dst = zT_dst[:, j, h * 512:(h + 1) * 512]
            if on_scalar:
                nc.scalar.copy(dst, pt[:])
            else:
                nc.vector.tensor_copy(dst, pt[:])

    # ============ Phase A: x load, LN1, transpose, V-proj interleaved ========
    zT_t = bigp.tile([128, 8192], F8, name="zT_all", tag="big")
    zT = zT_t[:].rearrange("p (d t) -> p d t", t=1024)
    V65 = [v65p.tile([128, 2080], F8, name=f"v65_{i}", tag="v65")
           for i in range(4)]
    with tc.tile_pool(name="ps_a", bufs=2, space="PSUM") as ps_a, \
         tc.tile_pool(name="ps_v", bufs=2, space="PSUM") as ps_v:
        wvp = []
        for dp in range(4):
            wt = wvop.tile([128, 2048], F8, name=f"wvp_{dp}", tag="wvo")
            src = din["wv"][dp * 256:(dp + 1) * 256, :]
            nc.sync.dma_start(
                wt[:].rearrange("p (two m) -> p two m", m=1024),
                src.rearrange("(two p) m -> p two m", p=128))
            wvp.append(wt)

        def vproj_t(t):
            pt = ps_v.tile([128, 1024], F32, tag="psv")
            if with_bias:
                for c in range(2):
                    bs = bias_slice(din["bv"], c * 512, 512)
                    nc.tensor.matmul(pt[:, c * 512:(c + 1) * 512],
                                     ones[0:1, 0:128], bs,
                                     start=True, stop=False)
            for dp in range(4):
                for c in range(2):
                    nc.tensor.matmul(
                        pt[:, c * 512:(c + 1) * 512],
                        zT[:, 2 * dp:2 * dp + 2, t * 128:(t + 1) * 128],
                        wvp[dp][:].rearrange("p (two m) -> p two m", m=1024)
                        [:, :, c * 512:(c + 1) * 512],
                        start=(dp == 0 and not with_bias),
                        stop=(dp == 3), perf_mode=DR)
            v3 = V65[t // 2][:].rearrange("p (two f) -> p two f", f=1040)
            pv = pt[:].rearrange("p (h c) -> p h c", c=64)
            dv = v3[:, t % 2, :].rearrange("p (h c) -> p h c", c=65)[:, :, 0:64]
            nc.vector.tensor_scalar_mul(dv, pv, 1.0 / SCL_V)
            oc = v3[:, t % 2, :].rearrange("p (h c) -> p h c", c=65)[:, :, 64:65]
            nc.gpsimd.partition_broadcast(
                oc, onescol[:].rearrange("p (h c) -> p h c", c=1))

        z_tiles = []
        for t in range(ST):
            xt = xp.tile([128, 1024], F32, tag="x")
            nc.sync.dma_start(xt[:], din["x"][t * 128:(t + 1) * 128, :])
            z_tiles.append(ln_stats_z(xt))
        # zT copies on the scalar engine: vector is the phase-A bottleneck.
        # V-proj of each token-half starts as soon as its transposes land.
        transpose_half(z_tiles, 0, zT, ps_a, on_scalar=True)
        for t in range(4):
            vproj_t(t)
        transpose_half(z_tiles, 1, zT, ps_a, on_scalar=True)
        for t in range(4, 8):
            vproj_t(t)
    if "dbg_zT" in ddbg:
        nc.sync.dma_start(ddbg["dbg_zT"], zT_t[:])
    if "dbg_v65" in ddbg:
        for i in range(4):
            nc.sync.dma_start(ddbg["dbg_v65"][i], V65[i][:])

    # ==================== Phase B: attention pass c=0 ========================
    ctxT_t = bigp.tile([128, 8192], F8, name="ctxT_all", tag="big")
    ctxT = ctxT_t[:].rearrange("p (d t) -> p d t", t=1024)

    # prefetch wo pairs now (reuses the wv buffers; consumed in phase C/D)
    wop = []
    for dp in range(4):
        wt = wvop.tile([128, 2048], F8, name=f"wop_{dp}", tag="wvo")
        src = din["wo"][dp * 256:(dp + 1) * 256, :]
        nc.sync.dma_start(
            wt[:].rearrange("p (two m) -> p two m", m=1024),
            src.rearrange("(two p) m -> p two m", p=128))
        wop.append(wt)

    # ---- attention machinery (used by both passes) ----
    Qs = [None] * 8
    Ks = [None] * 8

    ps_sc_ctx = tc.tile_pool(name="ps_sc", bufs=2, space="PSUM")
    ps_sc = ps_sc_ctx.__enter__()
    ps_pav_ctx = tc.tile_pool(name="ps_pav", bufs=2, space="PSUM")
    ps_pav = ps_pav_ctx.__enter__()
    ps_pj_ctx = tc.tile_pool(name="ps_pj", bufs=2, space="PSUM")
    ps_pj = ps_pj_ctx.__enter__()

    def proj_units(hp):
        """Q/K projection for head-pair hp as 4 filler units (wq/wk x c)."""
        state = {}

        def chunk(dw, dbias, scl, lst, c):
            if c == 0:
                wcol = wp.tile([128, 1024], F8, tag="w", name=f"w_{dw}_{hp}")
                src = din[dw][:, hp * 128:(hp + 1) * 128]
                src = src.rearrange("(dt p) m -> p dt m", p=128)
                wc3 = wcol[:].rearrange("p (dt m) -> p dt m", m=128)
                nc.sync.dma_start(wc3, src)
                ot = qkp.tile([128, 1024], F8, tag="qk", name=f"qk_{dw}_{hp}")
                state[dw] = (wc3, ot)
                lst[hp] = ot
            wc3, ot = state[dw]
            p = ps_pj.tile([128, 512], F32, tag="pj")
            if with_bias:
                bs = bias_slice(din[dbias], hp * 128, 128)
                nc.tensor.matmul(p[:], bs, ones[0:1, 0:512],
                                 start=True, stop=False)
            for dp in range(4):
                nc.tensor.matmul(
                    p[:],
                    wc3[:, 2 * dp:2 * dp + 2, :],
                    zT[:, 2 * dp:2 * dp + 2, c * 512:(c + 1) * 512],
                    start=(dp == 0 and not with_bias),
                    stop=(dp == 3), perf_mode=DR)
            nc.vector.tensor_scalar_mul(ot[:, c * 512:(c + 1) * 512], p[:],
                                        1.0 / scl)

        us = []
        for dw, dbias, scl, lst in (("wq", "bq", SCL_Q / QF8, Qs),
                                    ("wk", "bk", SCL_K, Ks)):
            for c in range(2):
                us.append(lambda dw=dw, dbias=dbias, scl=scl, lst=lst, c=c:
                          chunk(dw, dbias, scl, lst, c))
        return us

    def new_exp3():
        out = []
        for _ in range(4):
            et = expp.tile([128, 2048], F8, tag="exp", name=f"e_{nc.next_id()}")
            out.append(et[:].rearrange("p (two f) -> p two f", f=1024))
        return out

    def emit_scores_ktp(hp, c, ktp, exp3):
        """Scores + exp for key tiles 2*ktp, 2*ktp+1 of (hp, query-half c)."""
        QTh, KTh = Qs[hp], Ks[hp]
        for kt in (2 * ktp, 2 * ktp + 1):
            sc = ps_sc.tile([128, 1024], F32, tag="pss")
            nc.tensor.matmul(sc[:, 0:512],
                             KTh[0:64, kt * 128:(kt + 1) * 128],
                             QTh[0:64, c * 512:(c + 1) * 512],
                             start=True, stop=True)
            nc.tensor.matmul(sc[:, 512:1024],
                             KTh[64:128, kt * 128:(kt + 1) * 128],
                             QTh[64:128, c * 512:(c + 1) * 512],
                             start=True, stop=True)
            nc.scalar.activation(exp3[kt // 2][:, kt % 2, :], sc[:],
                                 AF.Exp, bias=neg2_t[:], scale=1.0 / QF8)

    def emit_av_tail(hp, c, exp3, bcast_pool=None):
        """AV (fp8 DR, kt pairs) + softmax tail for (hp, query-half c).

        bcast_pool: if given, the reciprocal-denominator broadcast across
        partitions runs as two tiny PE matmuls into that psum pool
        (offloads the vector engine); else 4 DVE stream_shuffles.
        """
        pavA = ps_pav.tile([128, 512], F32, tag="psav")
        pavB = ps_pav.tile([128, 512], F32, tag="psav")
        for ktp in range(4):
            v3 = V65[ktp][:].rearrange("p (two f) -> p two f", f=1040)
            e3 = exp3[ktp]
            nc.tensor.matmul(pavA[0:65, :],
                             v3[:, :, (2 * hp) * 65:(2 * hp) * 65 + 65],
                             e3[:, :, 0:512],
                             start=(ktp == 0), stop=(ktp == 3), perf_mode=DR)
            nc.tensor.matmul(pavB[0:65, :],
                             v3[:, :, (2 * hp + 1) * 65:(2 * hp + 1) * 65 + 65],
                             e3[:, :, 512:1024],
                             start=(ktp == 0), stop=(ktp == 3), perf_mode=DR)
        cslice = slice(c * 512, (c + 1) * 512)
        psum_pair = attp.tile([128, 512], F32, tag="psum_pair")
        if bcast_pool is not None:
            nc.vector.tensor_copy(psum_pair[0:1, :], pavA[64:65, :])
            nc.vector.tensor_copy(psum_pair[32:33, :], pavB[64:65, :])
            bps = bcast_pool.tile([128, 512], F32, tag="pj")
            nc.tensor.matmul(bps[0:64, :], ones2f[0:1, :],
                             psum_pair[0:1, :], start=True, stop=True)
            nc.tensor.matmul(bps[64:128, :], ones2f[32:33, :],
                             psum_pair[32:33, :], start=True, stop=True)
            rbc = rbcp.tile([128, 512], F32, tag="rbc")
            pscr = scrp.tile([128, 512], F32, tag="pscr")
            nc.vector.reciprocal_approx_accurate(rbc[:], bps[:], pscr[:])
        else:
            nc.vector.tensor_copy(psum_pair[0:1, :], pavA[64:65, :])
            nc.vector.tensor_copy(psum_pair[32:33, :], pavB[64:65, :])
            prec = scrp.tile([128, 512], F32, tag="prec")
            pscr = scrp.tile([128, 512], F32, tag="pscr")
            nc.vector.reciprocal_approx_accurate(prec[:], psum_pair[:], pscr[:])
            rbc = rbcp.tile([128, 512], F32, tag="rbc")
            bmask = [0] * 32
            nc.vector.stream_shuffle(rbc[0:32, :], prec[0:32, :], bmask)
            nc.vector.stream_shuffle(rbc[32:64, :], prec[0:32, :], bmask)
            nc.vector.stream_shuffle(rbc[64:96, :], prec[32:64, :], bmask)
            nc.vector.stream_shuffle(rbc[96:128, :], prec[32:64, :], bmask)
        nc.vector.scalar_tensor_tensor(
            ctxT[0:64, hp, cslice], pavA[0:64, :], SCL_CTX, rbc[0:64, :],
            op0=MULT, op1=MULT)
        nc.vector.scalar_tensor_tensor(
            ctxT[64:128, hp, cslice], pavB[0:64, :], SCL_CTX, rbc[64:128, :],
            op0=MULT, op1=MULT)

    # ---- attention pass c=0: QK projection chunks fill the exp waits ----
    for u in proj_units(0):
        u()
    for hp in range(8):
        exp3 = new_exp3()
        pu = proj_units(hp + 1) if hp < 7 else []
        for ktp in range(4):
            emit_scores_ktp(hp, 0, ktp, exp3)
            if ktp < len(pu):
                pu[ktp]()
        emit_av_tail(hp, 0, exp3, bcast_pool=ps_pj)
    ps_pj_ctx.__exit__(None, None, None)

    # ==================== Phase C: attention c=1 || half-0 pipeline ==========
    x2_tiles = [None] * ST
    z2_tiles = [None] * ST
    xres_tiles = {}
    z2T = [mbf.tile([128, 1024], BF16, name=f"z2T_{j}", tag="mbf")
           for j in range(DT)]
    gu_h = {0: [], 1: []}
    pools = {}

    def outproj_tc(t, c, pool):
        """Out-proj for token tile t, D-column half c (1 psum bank)."""
        pt = pool.tile([128, 512], F32, tag="pso")
        if with_bias:
            bs = bias_slice(din["bo"], c * 512, 512)
            nc.tensor.matmul(pt[:], ones[0:1, 0:128], bs,
                             start=True, stop=False)
        for dp in range(4):
            nc.tensor.matmul(
                pt[:],
                ctxT[:, 2 * dp:2 * dp + 2, t * 128:(t + 1) * 128],
                wop[dp][:].rearrange("p (two m) -> p two m", m=1024)
                [:, :, c * 512:(c + 1) * 512],
                start=(dp == 0 and not with_bias),
                stop=(dp == 3), perf_mode=DR)
        if t not in xres_tiles:
            xres = xp.tile([128, 1024], F32, tag="x")
            nc.sync.dma_start(xres[:], din["x"][t * 128:(t + 1) * 128, :])
            xres_tiles[t] = xres
            x2_tiles[t] = x2p.tile([128, 1024], F32, tag="x2",
                                   name=f"x2_{t}")
        nc.vector.scalar_tensor_tensor(
            x2_tiles[t][:, c * 512:(c + 1) * 512], pt[:],
            1.0 / (SCL_CTX * SCL_O),
            xres_tiles[t][:, c * 512:(c + 1) * 512], op0=MULT, op1=ADD)
        if c == 1:
            z2_tiles[t] = ln_stats_z(x2_tiles[t])
            del xres_tiles[t]

    def ffn1_fp(hf, fp, pool):
        gt = mbf.tile([128, 1024], BF16, tag="mbf")
        for sub in range(2):
            ft = fp * 2 + sub
            wcol = wp.tile([128, 1024], BF16, tag="w")
            src = din["w1"][:, ft * 128:(ft + 1) * 128]
            src = src.rearrange("(dt p) m -> p dt m", p=128)
            dst = wcol[:].rearrange("p (dt m) -> p dt m", m=128)
            nc.sync.dma_start(dst, src)
            p = pool.tile([128, 512], F32, tag="psf1")
            if with_bias:
                bs = bias_slice(din["b1"], ft * 128, 128)
                nc.tensor.matmul(p[:], bs, ones[0:1, 0:512],
                                 start=True, stop=False)
            for d in range(DT):
                nc.tensor.matmul(
                    p[:],
                    wcol[:, d * 128:(d + 1) * 128],
                    z2T[d][:, hf * 512:(hf + 1) * 512],
                    start=(d == 0 and not with_bias), stop=(d == DT - 1))
            nc.scalar.activation(gt[:, sub * 512:(sub + 1) * 512], p[:],
                                 AF.Gelu)
        gu_h[hf].append(gt)

    # filler units for the c=1 pass: (name, cost_us, fn)
    units = []
    for t in range(4):
        for c in range(2):
            units.append((f"o_{t}_{c}", 1.0,
                          lambda t=t, c=c: outproj_tc(t, c, pools["oc"])))
    units.append(("swap", 0.1, None))   # close out-proj psum, open ffn1 pool
    units.append(("ln2", 4.0,
                  lambda: transpose_half(z2_tiles, 0, z2T, ps_pav,
                                         tag="psav")))
    for fp in range(FT // 2):
        units.append((f"f1_{fp}", 3.4,
                      lambda fp=fp: ffn1_fp(0, fp, pools["c2"])))
    total_cost = sum(u[1] for u in units)
    SLOT = total_cost / 32.0

    pools["oc_ctx"] = tc.tile_pool(name="ps_oc", bufs=2, space="PSUM")
    pools["oc"] = pools["oc_ctx"].__enter__()

    def emit_unit(u):
        nm, cost, fn = u
        if nm == "swap":
            pools["oc_ctx"].__exit__(None, None, None)
            pools["c2_ctx"] = tc.tile_pool(name="ps_c2", bufs=2, space="PSUM")
            pools["c2"] = pools["c2_ctx"].__enter__()
        else:
            fn()
        return cost

    ui = 0
    spent = 0.0
    exp3 = new_exp3()
    for hp in range(8):
        for ktp in range(4):
            emit_scores_ktp(hp, 1, ktp, exp3)
            slot = hp * 4 + ktp
            while ui < len(units) and spent < (slot + 1) * SLOT:
                spent += emit_unit(units[ui])
                ui += 1
        emit_av_tail(hp, 1, exp3)
        if hp < 7:
            exp3 = new_exp3()
    while ui < len(units):
        spent += emit_unit(units[ui])
        ui += 1
    if "dbg_ctxT" in ddbg:
        nc.sync.dma_start(ddbg["dbg_ctxT"], ctxT_t[:])

    # attention + half-0 psum pools done (LIFO order)
    pools["c2_ctx"].__exit__(None, None, None)
    ps_pav_ctx.__exit__(None, None, None)
    ps_sc_ctx.__exit__(None, None, None)

    # ==================== Phase D: FFN2-h0, out-proj h1, FFN h1 ==============
    def ffn2_half(hf, f2pool):
        for c in range(2):
            accs = [f2pool.tile([128, 512], F32, name=f"acc_{hf}_{c}_{i}",
                                tag="psf2") for i in range(4)]
            if with_bias:
                for tl in range(4):
                    bs = bias_slice(din["b2"], c * 512, 512)
                    nc.tensor.matmul(accs[tl][:], ones[0:1, 0:128], bs,
                                     start=True, stop=False)
            for ft in range(FT):
                w2t = w2p.tile([128, 512], BF16, tag="w2")
                nc.sync.dma_start(
                    w2t[:],
                    din["w2"][ft * 128:(ft + 1) * 128,
                              c * 512:(c + 1) * 512])
                for tl in range(4):
                    lo = (ft % 2) * 512 + tl * 128
                    nc.tensor.matmul(
                        accs[tl][:],
                        gu_h[hf][ft // 2][:, lo:lo + 128],
                        w2t[:],
                        start=(ft == 0 and not with_bias), stop=(ft == FT - 1))
            for tl in range(4):
                t = hf * 4 + tl
                ot = outp.tile([128, 512], F32, tag="outp")
                nc.vector.tensor_add(ot[:],
                                     x2_tiles[t][:, c * 512:(c + 1) * 512],
                                     accs[tl][:])
                nc.sync.dma_start(
                    d_out[t * 128:(t + 1) * 128, c * 512:(c + 1) * 512],
                    ot[:])

    with tc.tile_pool(name="ps_f2", bufs=4, space="PSUM") as ps_f2, \
         tc.tile_pool(name="ps_oc2", bufs=2, space="PSUM") as ps_oc2, \
         tc.tile_pool(name="ps_f1b", bufs=2, space="PSUM") as ps_f1b:
        ffn2_half(0, ps_f2)
        for t in range(4, 8):
            for c in range(2):
                outproj_tc(t, c, ps_oc2)
        transpose_half(z2_tiles, 1, z2T, ps_oc2, tag="pso")
        for fp in range(FT // 2):
            ffn1_fp(1, fp, ps_f1b)
        ffn2_half(1, ps_f2)
    if "dbg_x2" in ddbg:
        for t in range(ST):
            nc.sync.dma_start(ddbg["dbg_x2"][t * 128:(t + 1) * 128, :], x2_tiles[t][:])


def _get_program(with_bias, dbg=False):
    key = ("prog", with_bias, dbg)
    if key not in _CACHE:
        _CACHE[key] = _build_program(with_bias, dbg)
    return _CACHE[key]


def _prepare(x, Wq, bq, Wk, bk, Wv, bv, Wo, bo, W1, b1, W2, b2,
             g1, be1, g2, be2, dbg=False):
    x = np.asarray(x, dtype=np.float32)
    f64 = np.float64

    # Fold LN affine params into the following projections (exact algebra):
    # (z*g + be) @ W + b = z @ (g[:,None]*W) + (be @ W + b);
    # 1/sqrt(hd) folded into Wq/bq.
    scale_q = 1.0 / np.sqrt(np.float64(HD))
    wq_eff = (np.asarray(g1, f64)[:, None] * np.asarray(Wq, f64)) * scale_q
    bq_eff = (np.asarray(be1, f64) @ np.asarray(Wq, f64) + np.asarray(bq, f64)) * scale_q
    wk_eff = np.asarray(g1, f64)[:, None] * np.asarray(Wk, f64)
    bk_eff = np.asarray(be1, f64) @ np.asarray(Wk, f64) + np.asarray(bk, f64)
    wv_eff = np.asarray(g1, f64)[:, None] * np.asarray(Wv, f64)
    bv_eff = np.asarray(be1, f64) @ np.asarray(Wv, f64) + np.asarray(bv, f64)
    w1_eff = np.asarray(g2, f64)[:, None] * np.asarray(W1, f64)
    b1_eff = np.asarray(be2, f64) @ np.asarray(W1, f64) + np.asarray(b1, f64)

    biases = [bq_eff, bk_eff, bv_eff, np.asarray(bo, f64),
              b1_eff, np.asarray(b2, f64)]
    with_bias = any(np.any(b != 0.0) for b in biases)

    nc = _get_program(with_bias, dbg)

    common = {
        "wq": np.ascontiguousarray((wq_eff * SCL_Q).astype(F8NP)),
        "wk": np.ascontiguousarray((wk_eff * SCL_K).astype(F8NP)),
        "wv": np.ascontiguousarray((wv_eff * SCL_V).astype(F8NP)),
        "wo": np.ascontiguousarray((np.asarray(Wo, f64) * SCL_O).astype(F8NP)),
        "w1": np.ascontiguousarray(w1_eff.astype(BFNP)),
        "w2": np.ascontiguousarray(np.asarray(W2, f64).astype(BFNP)),
        # bias matmuls run in bf16 against already-scaled psums
        "bq": (bq_eff * SCL_Q).astype(BFNP).reshape(1, D),
        "bk": (bk_eff * SCL_K).astype(BFNP).reshape(1, D),
        "bv": (bv_eff * SCL_V).astype(BFNP).reshape(1, D),
        "bo": (np.asarray(bo, f64) * SCL_CTX * SCL_O).astype(BFNP).reshape(1, D),
        "b1": b1_eff.astype(BFNP).reshape(1, FF),
        "b2": np.asarray(b2, f64).astype(BFNP).reshape(1, D),
        "ident": np.eye(128, dtype=BFNP),
        "ones": np.ones((1, 512), dtype=BFNP),
        "onescol": np.ones((1, 16), dtype=F8NP),
    }
    in_maps = []
    for b in range(NCORES):
        m = dict(common)
        m["x"] = np.ascontiguousarray(x[b])
        in_maps.append(m)
    return nc, in_maps


def kernel(**inputs):
    nc, in_maps = _prepare(**inputs)
    res = bass_utils.run_bass_kernel_spmd(nc, in_maps,
                                          core_ids=list(range(NCORES)))
    out = np.stack([res.results[b]["out"] for b in range(NCORES)], axis=0)
    return out.astype(np.float32)


def _timed_run(inputs, trace_cores=None):
    """Test-harness helper: rerun with NTFF tracing to get HW exec time."""
    nc, in_maps = _prepare(**inputs)
    try:
        return bass_utils.run_bass_kernel_spmd(
            nc, in_maps, core_ids=list(range(NCORES)), trace=True,
            trace_cores=trace_cores)
    except Exception as e:
        print(f"traced run failed: {e}")
        return None


# revision 42
# speedup vs baseline: 1.0211x; 1.0211x over previous
"""Trainium2 Bass kernel for a pre-norm transformer encoder layer.

Problem: x(8,1024,1024) fp32; LN1 -> MHA(16 heads, hd=64) + residual;
LN2 -> FFN(4096, exact gelu) + residual.

Strategy:
- Data-parallel: one batch element per NeuronCore (8 cores, no collectives).
- Attention-side matmuls (QKV projections, AV, out-proj) in fp8e4m3 with
  perf_mode=DoubleRow (2 contraction planes per instruction). Scores in
  fp8 at bf16 rate, packed 2-heads-concurrent via PE row tiling (K=64).
  FFN stays bf16: fp8 there pushes max-err past the 2e-2 gate.
- fp8 operands pre-scaled by powers of 2 into e4m3's normal range;
  scales folded back exactly via the ACT scale arg, DVE copy multiplies,
  and the residual scalar_tensor_tensor.
- exp(scores - 2) on the scalar engine (softmax shift-invariance) keeps
  fp8 exp values << 240 (e4m3 max).
- Softmax denominator via [V | 1] augmented AV matmul (65th output row);
  normalization fused into one scalar_tensor_tensor per ctx half.
- Query-half pipelining: attention runs per query-half (c=0 then c=1).
  During the c=1 pass, out-proj / LN2 / FFN1 of query-half 0 are emitted
  as filler units between score steps, so the scalar-engine exp chain
  hides behind dense PE work (which also keeps the HAM clock-gate at
  K=8/8). Q^T/K^T for all 8 head-pairs persist in fp8 across the passes.
"""

import numpy as np
import ml_dtypes
from contextlib import ExitStack

import concourse.bass as bass
import concourse.tile as tile
import concourse.mybir as mybir
from concourse import bacc
from concourse import bass_utils

F32 = mybir.dt.float32
BF16 = mybir.dt.bfloat16
F8 = mybir.dt.float8e4
AF = mybir.ActivationFunctionType
DR = mybir.MatmulPerfMode.DoubleRow
MULT = mybir.AluOpType.mult
ADD = mybir.AluOpType.add

F8NP = ml_dtypes.float8_e4m3fn
BFNP = ml_dtypes.bfloat16

S, D, H, HD, FF = 1024, 1024, 16, 64, 4096
ST, DT, FT = S // 128, D // 128, FF // 128
EPS = 1e-5
NCORES = 8

# power-of-2 scales bringing fp8 operand std to ~1
SCL_Q = 256.0   # wq_eff has 1/sqrt(hd)=1/8 folded in: std 1/256
SCL_K = 32.0
SCL_V = 32.0
SCL_O = 32.0
SCL_CTX = 16.0  # ctx std ~0.06
QF8 = 8.0       # Q tiles hold 8*q_eff (std ~1); exp scale arg = 1/8

_CACHE = {}


def _build_program(with_bias, dbg=False):
    nc = bacc.Bacc("TRN2", target_bir_lowering=False, debug=False,
                   num_devices=NCORES)

    din = {}
    for name, shape, dt in [
        ("x", (S, D), F32),
        ("wq", (D, D), F8), ("wk", (D, D), F8), ("wv", (D, D), F8),
        ("wo", (D, D), F8),
        ("w1", (D, FF), BF16), ("w2", (FF, D), BF16),
        ("bq", (1, D), BF16), ("bk", (1, D), BF16), ("bv", (1, D), BF16),
        ("bo", (1, D), BF16), ("b1", (1, FF), BF16), ("b2", (1, D), BF16),
        ("ident", (128, 128), BF16),
        ("ones", (1, 512), BF16), ("onescol", (1, 16), F8),
    ]:
        din[name] = nc.dram_tensor(name, shape, dt, kind="ExternalInput").ap()
    d_out = nc.dram_tensor("out", (S, D), F32, kind="ExternalOutput").ap()
    ddbg = {}
    if dbg:
        for name, shape, dt in [
            ("dbg_zT", (128, 8192), F8), ("dbg_qT", (D, S), F8),
            ("dbg_kT", (D, S), F8), ("dbg_v65", (4, 128, 2080), F8),
            ("dbg_ctxT", (128, 8192), F8), ("dbg_x2", (S, D), F32),
        ]:
            ddbg[name] = nc.dram_tensor(name, shape, dt, kind="ExternalOutput").ap()

    with tile.TileContext(nc) as tc, ExitStack() as ctx:
        _body(nc, tc, ctx, din, d_out, with_bias, ddbg)
    nc.compile()
    return nc


def _body(nc, tc, ctx, din, d_out, with_bias, ddbg=None):
    ddbg = ddbg or {}
    xp = ctx.enter_context(tc.tile_pool(name="xp", bufs=3))       # x f32 transient
    x2p = ctx.enter_context(tc.tile_pool(name="x2p", bufs=8))     # x2 f32 resident
    zp = ctx.enter_context(tc.tile_pool(name="zp", bufs=6))       # z bf16 transient
    bigp = ctx.enter_context(tc.tile_pool(name="bigp", bufs=2))   # zT/ctxT f8 wide
    qkp = ctx.enter_context(tc.tile_pool(name="qkp", bufs=16))    # Q/K f8 persistent
    v65p = ctx.enter_context(tc.tile_pool(name="v65p", bufs=4))   # V65 pairs f8
    expp = ctx.enter_context(tc.tile_pool(name="expp", bufs=8))   # exp pairs f8
    mbf = ctx.enter_context(tc.tile_pool(name="mbf", bufs=24))    # z2T/gu bf16
    wp = ctx.enter_context(tc.tile_pool(name="wp", bufs=6))       # qk/w1 col stream
    wvop = ctx.enter_context(tc.tile_pool(name="wvop", bufs=4))   # wv/wo pairs f8
    w2p = ctx.enter_context(tc.tile_pool(name="w2p", bufs=8))
    outp = ctx.enter_context(tc.tile_pool(name="outp", bufs=2))
    smallp = ctx.enter_context(tc.tile_pool(name="smallp", bufs=4))
    cstp = ctx.enter_context(tc.tile_pool(name="cstp", bufs=1))
    attp = ctx.enter_context(tc.tile_pool(name="attp", bufs=1))
    scrp = ctx.enter_context(tc.tile_pool(name="scrp", bufs=1))
    rbcp = ctx.enter_context(tc.tile_pool(name="rbcp", bufs=1))
    biasp = ctx.enter_context(tc.tile_pool(name="biasp", bufs=4)) if with_bias else None

    # constants
    ident = cstp.tile([128, 128], BF16, tag="ident")
    nc.sync.dma_start(ident[:], din["ident"])
    ones = cstp.tile([1, 512], BF16, tag="ones")
    nc.sync.dma_start(ones[:], din["ones"])
    onescol = cstp.tile([1, 16], F8, tag="onescol")
    nc.sync.dma_start(onescol[:], din["onescol"])
    eps_t = cstp.tile([128, 1], F32, tag="eps")
    nc.vector.memset(eps_t[:], EPS)
    neg2_t = cstp.tile([128, 1], F32, tag="neg2")
    nc.vector.memset(neg2_t[:], -2.0)
    ones2f = cstp.tile([128, 64], F32, tag="ones2f")
    nc.vector.memset(ones2f[:], 1.0)

    def bias_slice(dsrc, lo, n):
        bt = biasp.tile([1, 512], BF16, tag="brow")
        nc.sync.dma_start(bt[0:1, 0:n], dsrc[0:1, lo:lo + n])
        return bt[0:1, 0:n]

    # ---------------- LayerNorm -> normalized z tile (token-major) ----------
    def ln_stats_z(xt):
        stats = smallp.tile([128, 2, 6], F32, tag="stats")
        nc.vector.bn_stats(stats[:, 0, :], xt[:, 0:512])
        nc.vector.bn_stats(stats[:, 1, :], xt[:, 512:1024])
        mv = smallp.tile([128, 2], F32, tag="mv")
        nc.vector.bn_aggr(mv[:], stats[:])
        std = smallp.tile([128, 1], F32, tag="std")
        nc.scalar.activation(std[:], mv[:, 1:2], AF.Sqrt, bias=eps_t[:])
        rstd = smallp.tile([128, 1], F32, tag="rstd")
        nc.vector.reciprocal(rstd[:], std[:])
        nmr = smallp.tile([128, 1], F32, tag="nmr")
        nc.vector.tensor_scalar(nmr[:], mv[:, 0:1], rstd[:], -1.0,
                                op0=MULT, op1=MULT)
        # normalize on the scalar engine: z = x*rstd + (-mu*rstd)
        zt = zp.tile([128, 1024], BF16, name=f"z_{nc.next_id()}", tag="zp")
        nc.scalar.activation(zt[:], xt[:], AF.Identity,
                             bias=nmr[:], scale=rstd[:])
        return zt

    def transpose_half(z_tiles, h, zT_dst, ps_pool, on_scalar=False,
                       tag="pst"):
        """Transpose token-half h of 4 z tiles into zT_dst[:, j, h*512:]."""
        for j in range(DT):
            pt = ps_pool.tile([128, 512], BF16, tag=tag)
            for tl in range(4):
                t = h * 4 + tl
                nc.tensor.transpose(pt[:, tl * 128:(tl + 1) * 128],
                                    z_tiles[t][:, j * 128:(j + 1) * 128],
                                    ident[:])
            if isinstance(zT_dst, list):
                dst = zT_dst[j][:, h * 512:(h + 1) * 512]
            else:
                dst = zT_dst[:, j, h * 512:(h + 1) * 512]
            if on_scalar:
                nc.scalar.copy(dst, pt[:])
            else:
                nc.vector.tensor_copy(dst, pt[:])

    # ============ Phase A: x load, LN1, transpose, V-proj interleaved ========
    zT_t = bigp.tile([128, 8192], F8, name="zT_all", tag="big")
    zT = zT_t[:].rearrange("p (d t) -> p d t", t=1024)
    V65 = [v65p.tile([128, 2080], F8, name=f"v65_{i}", tag="v65")
           for i in range(4)]
    with tc.tile_pool(name="ps_a", bufs=2, space="PSUM") as ps_a, \
         tc.tile_pool(name="ps_v", bufs=2, space="PSUM") as ps_v:
        wvp = []

        def vproj_t(t):
            pt = ps_v.tile([128, 1024], F32, tag="psv")
            if with_bias:
                for c in range(2):
                    bs = bias_slice(din["bv"], c * 512, 512)
                    nc.tensor.matmul(pt[:, c * 512:(c + 1) * 512],
                                     ones[0:1, 0:128], bs,
                                     start=True, stop=False)
            for dp in range(4):
                for c in range(2):
                    nc.tensor.matmul(
                        pt[:, c * 512:(c + 1) * 512],
                        zT[:, 2 * dp:2 * dp + 2, t * 128:(t + 1) * 128],
                        wvp[dp][:].rearrange("p (two m) -> p two m", m=1024)
                        [:, :, c * 512:(c + 1) * 512],
                        start=(dp == 0 and not with_bias),
                        stop=(dp == 3), perf_mode=DR)
            v3 = V65[t // 2][:].rearrange("p (two f) -> p two f", f=1040)
            pv = pt[:].rearrange("p (h c) -> p h c", c=64)
            dv = v3[:, t % 2, :].rearrange("p (h c) -> p h c", c=65)[:, :, 0:64]
            nc.vector.tensor_scalar_mul(dv, pv, 1.0 / SCL_V)
            oc = v3[:, t % 2, :].rearrange("p (h c) -> p h c", c=65)[:, :, 64:65]
            nc.gpsimd.partition_broadcast(
                oc, onescol[:].rearrange("p (h c) -> p h c", c=1))

        z_tiles = []
        for t in range(ST):
            xt = xp.tile([128, 1024], F32, tag="x")
            nc.sync.dma_start(xt[:], din["x"][t * 128:(t + 1) * 128, :])
            z_tiles.append(ln_stats_z(xt))
        for dp in range(4):
            wt = wvop.tile([128, 2048], F8, name=f"wvp_{dp}", tag="wvo")
            src = din["wv"][dp * 256:(dp + 1) * 256, :]
            nc.sync.dma_start(
                wt[:].rearrange("p (two m) -> p two m", m=1024),
                src.rearrange("(two p) m -> p two m", p=128))
            wvp.append(wt)
        # zT copies on the scalar engine: vector is the phase-A bottleneck.
        # V-proj of each token-half starts as soon as its transposes land.
        transpose_half(z_tiles, 0, zT, ps_a, on_scalar=True)
        for t in range(4):
            vproj_t(t)
        transpose_half(z_tiles, 1, zT, ps_a, on_scalar=True)
        for t in range(4, 8):
            vproj_t(t)
    if "dbg_zT" in ddbg:
        nc.sync.dma_start(ddbg["dbg_zT"], zT_t[:])
    if "dbg_v65" in ddbg:
        for i in range(4):
            nc.sync.dma_start(ddbg["dbg_v65"][i], V65[i][:])

    # ==================== Phase B: attention pass c=0 ========================
    ctxT_t = bigp.tile([128, 8192], F8, name="ctxT_all", tag="big")
    ctxT = ctxT_t[:].rearrange("p (d t) -> p d t", t=1024)

    # prefetch wo pairs now (reuses the wv buffers; consumed in phase C/D)
    wop = []
    for dp in range(4):
        wt = wvop.tile([128, 2048], F8, name=f"wop_{dp}", tag="wvo")
        src = din["wo"][dp * 256:(dp + 1) * 256, :]
        nc.sync.dma_start(
            wt[:].rearrange("p (two m) -> p two m", m=1024),
            src.rearrange("(two p) m -> p two m", p=128))
        wop.append(wt)

    # ---- attention machinery (used by both passes) ----
    Qs = [None] * 8
    Ks = [None] * 8

    ps_sc_ctx = tc.tile_pool(name="ps_sc", bufs=2, space="PSUM")
    ps_sc = ps_sc_ctx.__enter__()
    ps_pav_ctx = tc.tile_pool(name="ps_pav", bufs=2, space="PSUM")
    ps_pav = ps_pav_ctx.__enter__()
    ps_pj_ctx = tc.tile_pool(name="ps_pj", bufs=2, space="PSUM")
    ps_pj = ps_pj_ctx.__enter__()

    def proj_units(hp):
        """Q/K projection for head-pair hp as 4 filler units (wq/wk x c)."""
        state = {}

        def chunk(dw, dbias, scl, lst, c):
            if c == 0:
                wcol = wp.tile([128, 1024], F8, tag="w", name=f"w_{dw}_{hp}")
                src = din[dw][:, hp * 128:(hp + 1) * 128]
                src = src.rearrange("(dt p) m -> p dt m", p=128)
                wc3 = wcol[:].rearrange("p (dt m) -> p dt m", m=128)
                nc.sync.dma_start(wc3, src)
                ot = qkp.tile([128, 1024], F8, tag="qk", name=f"qk_{dw}_{hp}")
                state[dw] = (wc3, ot)
                lst[hp] = ot
            wc3, ot = state[dw]
            p = ps_pj.tile([128, 512], F32, tag="pj")
            if with_bias:
                bs = bias_slice(din[dbias], hp * 128, 128)
                nc.tensor.matmul(p[:], bs, ones[0:1, 0:512],
                                 start=True, stop=False)
            for dp in range(4):
                nc.tensor.matmul(
                    p[:],
                    wc3[:, 2 * dp:2 * dp + 2, :],
                    zT[:, 2 * dp:2 * dp + 2, c * 512:(c + 1) * 512],
                    start=(dp == 0 and not with_bias),
                    stop=(dp == 3), perf_mode=DR)
            nc.vector.tensor_scalar_mul(ot[:, c * 512:(c + 1) * 512], p[:],
                                        1.0 / scl)

        us = []
        for dw, dbias, scl, lst in (("wq", "bq", SCL_Q / QF8, Qs),
                                    ("wk", "bk", SCL_K, Ks)):
            for c in range(2):
                us.append(lambda dw=dw, dbias=dbias, scl=scl, lst=lst, c=c:
                          chunk(dw, dbias, scl, lst, c))
        return us

    def new_exp3():
        out = []
        for _ in range(4):
            et = expp.tile([128, 2048], F8, tag="exp", name=f"e_{nc.next_id()}")
            out.append(et[:].rearrange("p (two f) -> p two f", f=1024))
        return out

    def emit_scores_ktp(hp, c, ktp, exp3):
        """Scores + exp for key tiles 2*ktp, 2*ktp+1 of (hp, query-half c)."""
        QTh, KTh = Qs[hp], Ks[hp]
        for kt in (2 * ktp, 2 * ktp + 1):
            sc = ps_sc.tile([128, 1024], F32, tag="pss")
            nc.tensor.matmul(sc[:, 0:512],
                             KTh[0:64, kt * 128:(kt + 1) * 128],
                             QTh[0:64, c * 512:(c + 1) * 512],
                             start=True, stop=True)
            nc.tensor.matmul(sc[:, 512:1024],
                             KTh[64:128, kt * 128:(kt + 1) * 128],
                             QTh[64:128, c * 512:(c + 1) * 512],
                             start=True, stop=True)
            nc.scalar.activation(exp3[kt // 2][:, kt % 2, :], sc[:],
                                 AF.Exp, bias=neg2_t[:], scale=1.0 / QF8)

    def emit_av_tail(hp, c, exp3, bcast_pool=None):
        """AV (fp8 DR, kt pairs) + softmax tail for (hp, query-half c).

        bcast_pool: if given, the reciprocal-denominator broadcast across
        partitions runs as two tiny PE matmuls into that psum pool
        (offloads the vector engine); else 4 DVE stream_shuffles.
        """
        pavA = ps_pav.tile([128, 512], F32, tag="psav")
        pavB = ps_pav.tile([128, 512], F32, tag="psav")
        for ktp in range(4):
            v3 = V65[ktp][:].rearrange("p (two f) -> p two f", f=1040)
            e3 = exp3[ktp]
            nc.tensor.matmul(pavA[0:65, :],
                             v3[:, :, (2 * hp) * 65:(2 * hp) * 65 + 65],
                             e3[:, :, 0:512],
                             start=(ktp == 0), stop=(ktp == 3), perf_mode=DR)
            nc.tensor.matmul(pavB[0:65, :],
                             v3[:, :, (2 * hp + 1) * 65:(2 * hp + 1) * 65 + 65],
                             e3[:, :, 512:1024],
                             start=(ktp == 0), stop=(ktp == 3), perf_mode=DR)
        cslice = slice(c * 512, (c + 1) * 512)
        psum_pair = attp.tile([128, 512], F32, tag="psum_pair")
        if bcast_pool is not None:
            nc.vector.tensor_copy(psum_pair[0:1, :], pavA[64:65, :])
            nc.vector.tensor_copy(psum_pair[32:33, :], pavB[64:65, :])
            bps = bcast_pool.tile([128, 512], F32, tag="pj")
            nc.tensor.matmul(bps[0:64, :], ones2f[0:1, :],
                             psum_pair[0:1, :], start=True, stop=True)
            nc.tensor.matmul(bps[64:128, :], ones2f[32:33, :],
                             psum_pair[32:33, :], start=True, stop=True)
            rbc = rbcp.tile([128, 512], F32, tag="rbc")
            pscr = scrp.tile([128, 512], F32, tag="pscr")
            nc.vector.reciprocal_approx_accurate(rbc[:], bps[:], pscr[:])
        else:
            nc.vector.tensor_copy(psum_pair[0:1, :], pavA[64:65, :])
            nc.vector.tensor_copy(psum_pair[32:33, :], pavB[64:65, :])
            prec = scrp.tile([128, 512], F32, tag="prec")
            pscr = scrp.tile([128, 512], F32, tag="pscr")
            nc.vector.reciprocal_approx_accurate(prec[:], psum_pair[:], pscr[:])
            rbc = rbcp.tile([128, 512], F32, tag="rbc")
            bmask = [0] * 32
            nc.vector.stream_shuffle(rbc[0:32, :], prec[0:32, :], bmask)
            nc.vector.stream_shuffle(rbc[32:64, :], prec[0:32, :], bmask)
            nc.vector.stream_shuffle(rbc[64:96, :], prec[32:64, :], bmask)
            nc.vector.stream_shuffle(rbc[96:128, :], prec[32:64, :], bmask)
        nc.vector.scalar_tensor_tensor(
            ctxT[0:64, hp, cslice], pavA[0:64, :], SCL_CTX, rbc[0:64, :],
            op0=MULT, op1=MULT)
        nc.vector.scalar_tensor_tensor(
            ctxT[64:128, hp, cslice], pavB[0:64, :], SCL_CTX, rbc[64:128, :],
            op0=MULT, op1=MULT)

    # ---- attention pass c=0: QK projection chunks fill the exp waits ----
    for u in proj_units(0):
        u()
    for hp in range(8):
        exp3 = new_exp3()
        pu = proj_units(hp + 1) if hp < 7 else []
        for ktp in range(4):
            emit_scores_ktp(hp, 0, ktp, exp3)
            if ktp < len(pu):
                pu[ktp]()
        emit_av_tail(hp, 0, exp3, bcast_pool=ps_pj)
    ps_pj_ctx.__exit__(None, None, None)

    # ==================== Phase C: attention c=1 || half-0 pipeline ==========
    x2_tiles = [None] * ST
    z2_tiles = [None] * ST
    xres_tiles = {}
    z2T = [mbf.tile([128, 1024], BF16, name=f"z2T_{j}", tag="mbf")
           for j in range(DT)]
    gu_h = {0: [], 1: []}
    pools = {}

    def outproj_tc(t, c, pool):
        """Out-proj for token tile t, D-column half c (1 psum bank)."""
        pt = pool.tile([128, 512], F32, tag="pso")
        if with_bias:
            bs = bias_slice(din["bo"], c * 512, 512)
            nc.tensor.matmul(pt[:], ones[0:1, 0:128], bs,
                             start=True, stop=False)
        for dp in range(4):
            nc.tensor.matmul(
                pt[:],
                ctxT[:, 2 * dp:2 * dp + 2, t * 128:(t + 1) * 128],
                wop[dp][:].rearrange("p (two m) -> p two m", m=1024)
                [:, :, c * 512:(c + 1) * 512],
                start=(dp == 0 and not with_bias),
                stop=(dp == 3), perf_mode=DR)
        if t not in xres_tiles:
            xres = xp.tile([128, 1024], F32, tag="x")
            nc.sync.dma_start(xres[:], din["x"][t * 128:(t + 1) * 128, :])
            xres_tiles[t] = xres
            x2_tiles[t] = x2p.tile([128, 1024], F32, tag="x2",
                                   name=f"x2_{t}")
        nc.vector.scalar_tensor_tensor(
            x2_tiles[t][:, c * 512:(c + 1) * 512], pt[:],
            1.0 / (SCL_CTX * SCL_O),
            xres_tiles[t][:, c * 512:(c + 1) * 512], op0=MULT, op1=ADD)
        if c == 1:
            z2_tiles[t] = ln_stats_z(x2_tiles[t])
            del xres_tiles[t]

    def ffn1_fp(hf, fp, pool):
        gt = mbf.tile([128, 1024], BF16, tag="mbf")
        for sub in range(2):
            ft = fp * 2 + sub
            wcol = wp.tile([128, 1024], BF16, tag="w")
            src = din["w1"][:, ft * 128:(ft + 1) * 128]
            src = src.rearrange("(dt p) m -> p dt m", p=128)
            dst = wcol[:].rearrange("p (dt m) -> p dt m", m=128)
            nc.sync.dma_start(dst, src)
            p = pool.tile([128, 512], F32, tag="psf1")
            if with_bias:
                bs = bias_slice(din["b1"], ft * 128, 128)
                nc.tensor.matmul(p[:], bs, ones[0:1, 0:512],
                                 start=True, stop=False)
            for d in range(DT):
                nc.tensor.matmul(
                    p[:],
                    wcol[:, d * 128:(d + 1) * 128],
                    z2T[d][:, hf * 512:(hf + 1) * 512],
                    start=(d == 0 and not with_bias), stop=(d == DT - 1))
            nc.scalar.activation(gt[:, sub * 512:(sub + 1) * 512], p[:],
                                 AF.Gelu)
        gu_h[hf].append(gt)

    # filler units for the c=1 pass: (name, cost_us, fn)
    units = []
    for t in range(4):
        for c in range(2):
            units.append((f"o_{t}_{c}", 1.0,
                          lambda t=t, c=c: outproj_tc(t, c, pools["oc"])))
    units.append(("swap", 0.1, None))   # close out-proj psum, open ffn1 pool
    units.append(("ln2", 4.0,
                  lambda: transpose_half(z2_tiles, 0, z2T, ps_pav,
                                         tag="psav")))
    for fp in range(FT // 2):
        units.append((f"f1_{fp}", 3.4,
                      lambda fp=fp: ffn1_fp(0, fp, pools["c2"])))
    total_cost = sum(u[1] for u in units)
    SLOT = total_cost / 32.0

    pools["oc_ctx"] = tc.tile_pool(name="ps_oc", bufs=2, space="PSUM")
    pools["oc"] = pools["oc_ctx"].__enter__()

    def emit_unit(u):
        nm, cost, fn = u
        if nm == "swap":
            pools["oc_ctx"].__exit__(None, None, None)
            pools["c2_ctx"] = tc.tile_pool(name="ps_c2", bufs=2, space="PSUM")
            pools["c2"] = pools["c2_ctx"].__enter__()
        else:
            fn()
        return cost

    ui = 0
    spent = 0.0
    exp3 = new_exp3()
    for hp in range(8):
        for ktp in range(4):
            emit_scores_ktp(hp, 1, ktp, exp3)
            slot = hp * 4 + ktp
            while ui < len(units) and spent < (slot + 1) * SLOT:
                spent += emit_unit(units[ui])
                ui += 1
        emit_av_tail(hp, 1, exp3)
        if hp < 7:
            exp3 = new_exp3()
    while ui < len(units):
        spent += emit_unit(units[ui])
        ui += 1
    if "dbg_ctxT" in ddbg:
        nc.sync.dma_start(ddbg["dbg_ctxT"], ctxT_t[:])

    # attention + half-0 psum pools done (LIFO order)
    pools["c2_ctx"].__exit__(None, None, None)
    ps_pav_ctx.__exit__(None, None, None)
    ps_sc_ctx.__exit__(None, None, None)

    # ==================== Phase D: FFN2-h0, out-proj h1, FFN h1 ==============
    def ffn2_half(hf, f2pool):
        for c in range(2):
            accs = [f2pool.tile([128, 512], F32, name=f"acc_{hf}_{c}_{i}",
                                tag="psf2") for i in range(4)]
            if with_bias:
                for tl in range(4):
                    bs = bias_slice(din["b2"], c * 512, 512)
                    nc.tensor.matmul(accs[tl][:], ones[0:1, 0:128], bs,
                                     start=True, stop=False)
            for ft in range(FT):
                w2t = w2p.tile([128, 512], BF16, tag="w2")
                nc.sync.dma_start(
                    w2t[:],
                    din["w2"][ft * 128:(ft + 1) * 128,
                              c * 512:(c + 1) * 512])
                for tl in range(4):
                    lo = (ft % 2) * 512 + tl * 128
                    nc.tensor.matmul(
                        accs[tl][:],
                        gu_h[hf][ft // 2][:, lo:lo + 128],
                        w2t[:],
                        start=(ft == 0 and not with_bias), stop=(ft == FT - 1))
            for tl in range(4):
                t = hf * 4 + tl
                ot = outp.tile([128, 512], F32, tag="outp")
                nc.vector.tensor_add(ot[:],
                                     x2_tiles[t][:, c * 512:(c + 1) * 512],
                                     accs[tl][:])
                nc.sync.dma_start(
                    d_out[t * 128:(t + 1) * 128, c * 512:(c + 1) * 512],
                    ot[:])

    with tc.tile_pool(name="ps_f2", bufs=4, space="PSUM") as ps_f2, \
         tc.tile_pool(name="ps_oc2", bufs=2, space="PSUM") as ps_oc2, \
         tc.tile_pool(name="ps_f1b", bufs=2, space="PSUM") as ps_f1b:
        for t in range(4, 8):
            for c in range(2):
                outproj_tc(t, c, ps_oc2)
        transpose_half(z2_tiles, 1, z2T, ps_oc2, tag="pso")
        ffn2_half(0, ps_f2)
        for fp in range(FT // 2):
            ffn1_fp(1, fp, ps_f1b)
        ffn2_half(1, ps_f2)
    if "dbg_x2" in ddbg:
        for t in range(ST):
            nc.sync.dma_start(ddbg["dbg_x2"][t * 128:(t + 1) * 128, :], x2_tiles[t][:])


def _get_program(with_bias, dbg=False):
    key = ("prog", with_bias, dbg)
    if key not in _CACHE:
        _CACHE[key] = _build_program(with_bias, dbg)
    return _CACHE[key]


def _prepare(x, Wq, bq, Wk, bk, Wv, bv, Wo, bo, W1, b1, W2, b2,
             g1, be1, g2, be2, dbg=False):
    x = np.asarray(x, dtype=np.float32)
    f64 = np.float64

    # Fold LN affine params into the following projections (exact algebra):
    # (z*g + be) @ W + b = z @ (g[:,None]*W) + (be @ W + b);
    # 1/sqrt(hd) folded into Wq/bq.
    scale_q = 1.0 / np.sqrt(np.float64(HD))
    wq_eff = (np.asarray(g1, f64)[:, None] * np.asarray(Wq, f64)) * scale_q
    bq_eff = (np.asarray(be1, f64) @ np.asarray(Wq, f64) + np.asarray(bq, f64)) * scale_q
    wk_eff = np.asarray(g1, f64)[:, None] * np.asarray(Wk, f64)
    bk_eff = np.asarray(be1, f64) @ np.asarray(Wk, f64) + np.asarray(bk, f64)
    wv_eff = np.asarray(g1, f64)[:, None] * np.asarray(Wv, f64)
    bv_eff = np.asarray(be1, f64) @ np.asarray(Wv, f64) + np.asarray(bv, f64)
    w1_eff = np.asarray(g2, f64)[:, None] * np.asarray(W1, f64)
    b1_eff = np.asarray(be2, f64) @ np.asarray(W1, f64) + np.asarray(b1, f64)

    biases = [bq_eff, bk_eff, bv_eff, np.asarray(bo, f64),
              b1_eff, np.asarray(b2, f64)]
    with_bias = any(np.any(b != 0.0) for b in biases)

    nc = _get_program(with_bias, dbg)

    common = {
        "wq": np.ascontiguousarray((wq_eff * SCL_Q).astype(F8NP)),
        "wk": np.ascontiguousarray((wk_eff * SCL_K).astype(F8NP)),
        "wv": np.ascontiguousarray((wv_eff * SCL_V).astype(F8NP)),
        "wo": np.ascontiguousarray((np.asarray(Wo, f64) * SCL_O).astype(F8NP)),
        "w1": np.ascontiguousarray(w1_eff.astype(BFNP)),
        "w2": np.ascontiguousarray(np.asarray(W2, f64).astype(BFNP)),
        # bias matmuls run in bf16 against already-scaled psums
        "bq": (bq_eff * SCL_Q).astype(BFNP).reshape(1, D),
        "bk": (bk_eff * SCL_K).astype(BFNP).reshape(1, D),
        "bv": (bv_eff * SCL_V).astype(BFNP).reshape(1, D),
        "bo": (np.asarray(bo, f64) * SCL_CTX * SCL_O).astype(BFNP).reshape(1, D),
        "b1": b1_eff.astype(BFNP).reshape(1, FF),
        "b2": np.asarray(b2, f64).astype(BFNP).reshape(1, D),
        "ident": np.eye(128, dtype=BFNP),
        "ones": np.ones((1, 512), dtype=BFNP),
        "onescol": np.ones((1, 16), dtype=F8NP),
    }
    in_maps = []
    for b in range(NCORES):
        m = dict(common)
        m["x"] = np.ascontiguousarray(x[b])
        in_maps.append(m)
    return nc, in_maps


def kernel(**inputs):
    nc, in_maps = _prepare(**inputs)
    res = bass_utils.run_bass_kernel_spmd(nc, in_maps,
                                          core_ids=list(range(NCORES)))
    out = np.stack([res.results[b]["out"] for b in range(NCORES)], axis=0)
    return out.astype(np.float32)


def _timed_run(inputs, trace_cores=None):
    """Test-harness helper: rerun with NTFF tracing to get HW exec time."""
    nc, in_maps = _prepare(**inputs)
    try:
        return bass_utils.run_bass_kernel_spmd(
            nc, in_maps, core_ids=list(range(NCORES)), trace=True,
            trace_cores=trace_cores)
    except Exception as e:
        print(f"traced run failed: {e}")
        return None
